# revision 27
# baseline (speedup 1.0000x reference)
"""PointClickLoss Trainium2 kernel.

Reference computes, for each of 32 images (1024x1024), bilinear samples at
16 positive + 16 negative points, BCE-with-logits losses via softplus, and
a mean over points/images.  Only 4 taps per point are actually needed from
the 128 MB pred_mask, so the kernel is a tiny data-dependent gather:

  - data parallel over 8 NeuronCores, 4 images each
  - per core: 128 points -> one partition each (pos points in partitions
    0..63, neg in 64..127; host concatenates the two point tensors)
  - tap indices computed on-device (floor via int32 roundtrip + compare
    correction; HW CAST rounds to nearest), then two indirect DMAs of 128
    descriptors x 8 bytes (one per tap row; the HW DGE emits one descriptor
    per offset-AP partition, sized by the dest row - probed on HW)
  - bilinear combine + softplus(+/-z)=ln(1+exp(+/-z)) on DVE/ACT (both in
    the natural_log_exp_and_others table set - no table reload),
    partition-sum via PE matmul against a 1/1024-scaled ones vector
  - each core emits its partial; host adds the 8 scalars
"""

import sys

if "/opt/trn_rl_repo" not in sys.path:
    sys.path.insert(0, "/opt/trn_rl_repo")

import numpy as np

B, H, W = 32, 1024, 1024
NPOS = NNEG = 16
NCORES = 8
BL = B // NCORES          # images per core
P = 128                   # partitions used = BL * (NPOS + NNEG)

_BUILT = None
LAST_RESULTS = None       # BassKernelResults of the most recent run (for test.py)


def build():
    import concourse.bass as bass
    import concourse.mybir as mybir
    import concourse.tile as tile
    from concourse import bacc

    f32 = mybir.dt.float32
    i32 = mybir.dt.int32
    mult = mybir.AluOpType.mult
    add = mybir.AluOpType.add
    bypass = mybir.AluOpType.bypass

    orig_dab = _slim_drain_and_barrier(tile)
    # The two all-engine barriers Bass.__init__ emits include per-engine
    # DRAINs (the Sync one costs ~0.7us scanning idle DMA rings).  Nothing
    # is in flight at model entry (NRT's own entry barrier precedes us), so
    # sequencer-level event sync is enough - keep the ordering, drop the
    # drains.
    orig_aeb = bass.Bass.all_engine_barrier
    ncalls = [0]

    def _sem_only_aeb(self, **kw):
        ncalls[0] += 1
        if ncalls[0] == 2:
            return  # second init barrier isolates host-side bookkeeping only
        return orig_aeb(self, sem_only=True)

    bass.Bass.all_engine_barrier = _sem_only_aeb
    try:
        nc = bacc.Bacc()
    finally:
        bass.Bass.all_engine_barrier = orig_aeb
    pm = nc.dram_tensor("pm", [BL * H * W, 1], f32, kind="ExternalInput")
    pts_d = nc.dram_tensor("pts", [P, 2], f32, kind="ExternalInput")
    out_d = nc.dram_tensor("out", [1, 1], f32, kind="ExternalOutput")

    # Per-partition constants: partition p < 64 is positive point (p % 16) of
    # image p // 16; partition p >= 64 is negative point of image (p-64)//16.
    # cols: [flat base offset of the image, softplus sign, 1/1024, one, zero]
    pidx = np.arange(P)
    img = (pidx % 64) // 16
    base = (img * H * W).astype(np.float64)
    cb_np = np.stack(
        [
            base,
            base + W,
            np.where(pidx < 64, -1.0, 1.0),
            np.full(P, 1.0 / float(B * (NPOS + NNEG))),
            np.ones(P),
            np.zeros(P),
        ],
        axis=1,
    ).astype(np.float32)
    cb_d = nc.inline_tensor(cb_np, name="cb_const")

    with tile.TileContext(nc) as tc:
        with (
            tc.tile_pool(name="sb", bufs=1) as pool,
            tc.tile_pool(name="ps", bufs=1, space="PSUM") as psum,
        ):
            pts = pool.tile([P, 2], f32)          # (x, y) per point
            nc.sync.dma_start(pts[:, :], pts_d[:, :])
            cb = pool.tile([P, 6], f32)
            nc.scalar.dma_start(cb[:], cb_d[:, :])

            # --- critical index chain (4 DVE ops) ---
            # floor(x) = round_to_nearest(x - c) with c = largest f32 < 0.5.
            # HW casts round to nearest-even; exhaustively checked over
            # [0, 1023): result is floor(x), or floor(x)-1 exactly when the
            # frac weight comes out exactly 1.0 (bilinear then reads only the
            # in-range upper tap, so the sample is still exact).  CoreSim
            # truncates instead - sim_check models that separately.
            with tc.high_priority():
                xi = pool.tile([P, 2], i32)
                nc.vector.tensor_scalar(
                    out=xi[:], in0=pts[:], scalar1=-0.49999997, scalar2=None,
                    op0=add,
                )
                xy0 = pool.tile([P, 2], f32)      # (x0, y0) as floats (exact)
                nc.vector.tensor_copy(xy0[:], xi[:])
                r = pool.tile([P, 1], f32)        # y0*W + x0  (exact in f32)
                nc.vector.scalar_tensor_tensor(
                    out=r[:], in0=xy0[:, 1:2], scalar=float(W), in1=xy0[:, 0:1],
                    op0=mult, op1=add,
                )
                idx = pool.tile([P, 2], i32)      # +image base / +base+W
                nc.vector.scalar_tensor_tensor(
                    out=idx[:], in0=cb[:, 0:2], scalar=r[:, 0:1], in1=cb[:, 0:2],
                    op0=add, op1=bypass,
                )

            # Gather the 2x2 taps: one descriptor per partition per DMA, each
            # reading 2 contiguous f32 -> gv = (v00, v01, v10, v11).  The HW
            # DGE takes one offset per offset-AP *partition* sized by the dest
            # row (probed), so the two tap rows need two indirect DMAs.
            gv = pool.tile([P, 4], f32)
            nc.gpsimd.indirect_dma_start(
                out=gv[:, 0:2],
                out_offset=None,
                in_=pm[:, :],
                in_offset=bass.IndirectOffsetOnAxis(ap=idx[:, 0:1], axis=0),
            )
            nc.gpsimd.indirect_dma_start(
                out=gv[:, 2:4],
                out_offset=None,
                in_=pm[:, :],
                in_offset=bass.IndirectOffsetOnAxis(ap=idx[:, 1:2], axis=0),
            )

            # --- off-critical weight math (overlaps the gather) ---
            fr = pool.tile([P, 2], f32)           # (wx1, wy1)
            nc.vector.tensor_sub(fr[:], pts[:], xy0[:])
            g1 = pool.tile([P, 2], f32)           # (wx0, wy0) = 1 - frac
            nc.vector.scalar_tensor_tensor(
                out=g1[:], in0=xy0[:], scalar=1.0, in1=pts[:],
                op0=add, op1=mybir.AluOpType.subtract,
            )
            w4 = pool.tile([P, 4], f32)           # (wy0wx0, wy0wx1, wy1wx0, wy1wx1)
            nc.vector.tensor_tensor(w4[:, 0:1], g1[:, 1:2], g1[:, 0:1], mult)
            nc.vector.tensor_tensor(w4[:, 1:2], g1[:, 1:2], fr[:, 0:1], mult)
            nc.vector.tensor_tensor(w4[:, 2:3], fr[:, 1:2], g1[:, 0:1], mult)
            nc.vector.tensor_tensor(w4[:, 3:4], fr[:, 1:2], fr[:, 0:1], mult)

            # val[p] = sum of the 4 weighted taps
            tt = pool.tile([P, 4], f32)
            val = pool.tile([P, 1], f32)
            nc.vector.scalar_tensor_tensor(
                out=tt[:], in0=gv[:], scalar=1.0, in1=w4[:],
                op0=bypass, op1=mult, accum_out=val[:],
            )

            # loss[p] = softplus(sign[p] * val[p]) = ln(1 + exp(sign*val)).
            # bias/scale come from cb columns so no extra const DMAs appear.
            ez = pool.tile([P, 1], f32)
            nc.scalar.activation(
                out=ez[:], in_=val[:],
                func=mybir.ActivationFunctionType.Exp,
                bias=cb[:, 5:6], scale=cb[:, 2:3],
            )
            sp = pool.tile([P, 1], f32)
            nc.scalar.activation(
                out=sp[:], in_=ez[:],
                func=mybir.ActivationFunctionType.Ln,
                bias=cb[:, 4:5],
            )

            # partition sum via matmul against the 1/1024 column
            acc = psum.tile([1, 1], f32)
            nc.tensor.matmul(
                out=acc[:], lhsT=sp[:], rhs=cb[:, 3:4], start=True, stop=True
            )
            res = pool.tile([1, 1], f32)
            nc.vector.tensor_copy(res[:], acc[:])
            nc.sync.dma_start(out_d[:, :], res[:])

    tile.TileContext._drain_and_barrier = orig_dab
    _compile_with_joint_act_table(nc, bacc, mybir)
    return nc


def _slim_drain_and_barrier(tile_mod):
    """Kernel-exit: keep the drain (output DMA must land) + one all-engine
    barrier, but skip the end-of-kernel semaphore clear + second barrier -
    the Bass preamble re-clears all kernel semaphores at the start of every
    execution, so the exit clear only duplicates work on the measured path."""
    orig = tile_mod.TileContext._drain_and_barrier

    def slim(self, tick_clock, wait_clock):
        drain_inst = self.nc.sync.drain()
        wait_clock.add_sem_waits(
            drain_inst.ins, tile_mod.ScopedClock({None: tick_clock.global_clock})
        )
        self.nc.all_engine_barrier()
        popped = self.nc._tile_sem_poison_stack.pop()
        assert popped is self._sem_poison

    tile_mod.TileContext._drain_and_barrier = slim
    return orig


def _compile_with_joint_act_table(nc, bacc, mybir):
    """Compile with Exp and Ln forced into the one table set that holds both
    (natural_log_exp_and_others), avoiding a 1.3us ACT table reload between
    the two activations."""
    orig = bacc.get_activation_tables
    exp_t = mybir.ActivationFunctionType.Exp
    ln_t = mybir.ActivationFunctionType.Ln

    def patched(arch):
        tables = dict(orig(arch))
        joint = "natural_log_exp_and_others"
        if joint in tables and exp_t in tables[joint] and ln_t in tables[joint]:
            for name in tables:
                if name != joint:
                    tables[name] = tables[name] - {exp_t, ln_t}
        return tables

    bacc.get_activation_tables = patched
    try:
        nc.compile()
    finally:
        bacc.get_activation_tables = orig


def shard_inputs(pred_mask, positive_points, negative_points):
    pm = np.ascontiguousarray(
        np.asarray(pred_mask, dtype=np.float32).reshape(NCORES, BL * H * W, 1)
    )
    pos = np.asarray(positive_points, dtype=np.float32).reshape(NCORES, BL * NPOS, 2)
    neg = np.asarray(negative_points, dtype=np.float32).reshape(NCORES, BL * NNEG, 2)
    pts = np.ascontiguousarray(np.concatenate([pos, neg], axis=1))  # [8, 128, 2]
    return [{"pm": pm[c], "pts": pts[c]} for c in range(NCORES)]


def _harden_runtime():
    """Make run_bass_kernel_spmd robust in this container even if the caller
    enables tracing: the agent image's antenv lacks axon_hooks (crashes the
    trace path on import), and the S3 artifact upload has no credentials
    here.  Both shims are no-ops when tracing is off."""
    try:
        from antenv import axon_hooks  # noqa: F401
    except ImportError:
        import types

        import antenv

        mod = types.ModuleType("antenv.axon_hooks")
        _store = {}
        mod.set_axon_ntff_profile_hook = lambda h: _store.__setitem__("h", h)
        mod.get_axon_ntff_profile_hook = lambda: _store.get("h")
        sys.modules["antenv.axon_hooks"] = mod
        antenv.axon_hooks = mod
        try:
            from trn_agent_boot.trn_boot import _ntff_profile_via_ctypes

            mod.set_axon_ntff_profile_hook(
                _ntff_profile_via_ctypes("/opt/axon/libaxon_pjrt.so")
            )
        except Exception:
            pass

    from concourse import bass_utils

    if not getattr(bass_utils.upload_artifacts, "_failsafe", False):
        orig = bass_utils.upload_artifacts

        def safe_upload(tmpdir):
            try:
                return orig(tmpdir)
            except Exception:
                return f"file://{tmpdir}"

        safe_upload._failsafe = True
        bass_utils.upload_artifacts = safe_upload


def kernel(pred_mask, positive_points, negative_points):
    global _BUILT, LAST_RESULTS
    _harden_runtime()
    from concourse.bass_utils import run_bass_kernel_spmd

    if _BUILT is None:
        _BUILT = build()
    in_maps = shard_inputs(pred_mask, positive_points, negative_points)
    res = run_bass_kernel_spmd(_BUILT, in_maps, core_ids=list(range(NCORES)))
    LAST_RESULTS = res
    total = float(sum(float(r["out"][0, 0]) for r in res.results))
    return np.float32(total)


# revision 28
# speedup vs baseline: 1.2069x; 1.2069x over previous
"""PointClickLoss Trainium2 kernel.

Reference computes, for each of 32 images (1024x1024), bilinear samples at
16 positive + 16 negative points, BCE-with-logits losses via softplus, and
a mean over points/images.  Only 4 taps per point are actually needed from
the 128 MB pred_mask, so the kernel is a tiny data-dependent gather:

  - data parallel over 8 NeuronCores, 4 images each
  - per core: 128 points -> one partition each (pos points in partitions
    0..63, neg in 64..127; host concatenates the two point tensors)
  - tap indices computed on-device (floor via int32 roundtrip + compare
    correction; HW CAST rounds to nearest), then two indirect DMAs of 128
    descriptors x 8 bytes (one per tap row; the HW DGE emits one descriptor
    per offset-AP partition, sized by the dest row - probed on HW)
  - bilinear combine + softplus(+/-z)=ln(1+exp(+/-z)) on DVE/ACT (both in
    the natural_log_exp_and_others table set - no table reload),
    partition-sum via PE matmul against a 1/1024-scaled ones vector
  - each core emits its partial; host adds the 8 scalars
"""

import sys

if "/opt/trn_rl_repo" not in sys.path:
    sys.path.insert(0, "/opt/trn_rl_repo")

import numpy as np

B, H, W = 32, 1024, 1024
NPOS = NNEG = 16
NCORES = 8
BL = B // NCORES          # images per core
P = 128                   # partitions used = BL * (NPOS + NNEG)

_BUILT = None
LAST_RESULTS = None       # BassKernelResults of the most recent run (for test.py)


def build():
    import concourse.bass as bass
    import concourse.mybir as mybir
    import concourse.tile as tile
    from concourse import bacc

    f32 = mybir.dt.float32
    i32 = mybir.dt.int32
    mult = mybir.AluOpType.mult
    add = mybir.AluOpType.add
    bypass = mybir.AluOpType.bypass

    orig_dab = _slim_drain_and_barrier(tile)
    # The two all-engine barriers Bass.__init__ emits include per-engine
    # DRAINs (the Sync one costs ~0.7us scanning idle DMA rings).  Nothing
    # is in flight at model entry (NRT's own entry barrier precedes us), so
    # sequencer-level event sync is enough - keep the ordering, drop the
    # drains.
    orig_aeb = bass.Bass.all_engine_barrier
    ncalls = [0]

    def _sem_only_aeb(self, **kw):
        ncalls[0] += 1
        if ncalls[0] == 2:
            return  # second init barrier isolates host-side bookkeeping only
        return orig_aeb(self, sem_only=True)

    # Skip the four const-AP pool MEMSETs (const-float32-0.0 etc.) - this
    # kernel never reads them (bias/scale come from cb columns), and they
    # sit on Pool's serial path to the init barrier (~0.4us).
    orig_memset = bass.BassEitherVectorEngine.memset

    def _filtered_memset(self, ap, constant):
        name = getattr(ap.tensor, "name", "")
        if name.startswith("const-"):
            return None
        return orig_memset(self, ap, constant)

    bass.Bass.all_engine_barrier = _sem_only_aeb
    bass.BassEitherVectorEngine.memset = _filtered_memset
    try:
        nc = bacc.Bacc()
    finally:
        bass.Bass.all_engine_barrier = orig_aeb
        bass.BassEitherVectorEngine.memset = orig_memset
    pm = nc.dram_tensor("pm", [BL * H * W, 1], f32, kind="ExternalInput")
    pts_d = nc.dram_tensor("pts", [P, 2], f32, kind="ExternalInput")
    out_d = nc.dram_tensor("out", [1, 1], f32, kind="ExternalOutput")

    # Per-partition constants: partition p < 64 is positive point (p % 16) of
    # image p // 16; partition p >= 64 is negative point of image (p-64)//16.
    # cols: [flat base offset of the image, softplus sign, 1/1024, one, zero]
    pidx = np.arange(P)
    img = (pidx % 64) // 16
    base = (img * H * W).astype(np.float64)
    cb_np = np.stack(
        [
            base,
            base + W,
            np.where(pidx < 64, -1.0, 1.0),
            np.full(P, 1.0 / float(B * (NPOS + NNEG))),
            np.ones(P),
            np.zeros(P),
        ],
        axis=1,
    ).astype(np.float32)
    cb_d = nc.inline_tensor(cb_np, name="cb_const")

    with tile.TileContext(nc) as tc:
        with (
            tc.tile_pool(name="sb", bufs=1) as pool,
            tc.tile_pool(name="ps", bufs=1, space="PSUM") as psum,
        ):
            pts = pool.tile([P, 2], f32)          # (x, y) per point
            nc.sync.dma_start(pts[:, :], pts_d[:, :])
            cb = pool.tile([P, 6], f32)
            nc.scalar.dma_start(cb[:], cb_d[:, :])

            # --- critical index chain (4 DVE ops) ---
            # floor(x) = round_to_nearest(x - c) with c = largest f32 < 0.5.
            # HW casts round to nearest-even; exhaustively checked over
            # [0, 1023): result is floor(x), or floor(x)-1 exactly when the
            # frac weight comes out exactly 1.0 (bilinear then reads only the
            # in-range upper tap, so the sample is still exact).  CoreSim
            # truncates instead - sim_check models that separately.
            with tc.high_priority():
                xi = pool.tile([P, 2], i32)
                nc.vector.tensor_scalar(
                    out=xi[:], in0=pts[:], scalar1=-0.49999997, scalar2=None,
                    op0=add,
                )
                xy0 = pool.tile([P, 2], f32)      # (x0, y0) as floats (exact)
                nc.vector.tensor_copy(xy0[:], xi[:])
                r = pool.tile([P, 1], f32)        # y0*W + x0  (exact in f32)
                nc.vector.scalar_tensor_tensor(
                    out=r[:], in0=xy0[:, 1:2], scalar=float(W), in1=xy0[:, 0:1],
                    op0=mult, op1=add,
                )
                idx = pool.tile([P, 2], i32)      # +image base / +base+W
                nc.vector.scalar_tensor_tensor(
                    out=idx[:], in0=cb[:, 0:2], scalar=r[:, 0:1], in1=cb[:, 0:2],
                    op0=add, op1=bypass,
                )

            # Gather the 2x2 taps: one descriptor per partition per DMA, each
            # reading 2 contiguous f32 -> gv = (v00, v01, v10, v11).  The HW
            # DGE takes one offset per offset-AP *partition* sized by the dest
            # row (probed), so the two tap rows need two indirect DMAs.
            gv = pool.tile([P, 4], f32)
            nc.gpsimd.indirect_dma_start(
                out=gv[:, 0:2],
                out_offset=None,
                in_=pm[:, :],
                in_offset=bass.IndirectOffsetOnAxis(ap=idx[:, 0:1], axis=0),
            )
            nc.gpsimd.indirect_dma_start(
                out=gv[:, 2:4],
                out_offset=None,
                in_=pm[:, :],
                in_offset=bass.IndirectOffsetOnAxis(ap=idx[:, 1:2], axis=0),
            )

            # --- off-critical weight math (overlaps the gather) ---
            fr = pool.tile([P, 2], f32)           # (wx1, wy1)
            nc.vector.tensor_sub(fr[:], pts[:], xy0[:])
            g1 = pool.tile([P, 2], f32)           # (wx0, wy0) = 1 - frac
            nc.vector.scalar_tensor_tensor(
                out=g1[:], in0=xy0[:], scalar=1.0, in1=pts[:],
                op0=add, op1=mybir.AluOpType.subtract,
            )
            w4 = pool.tile([P, 4], f32)           # (wy0wx0, wy0wx1, wy1wx0, wy1wx1)
            nc.vector.tensor_tensor(w4[:, 0:1], g1[:, 1:2], g1[:, 0:1], mult)
            nc.vector.tensor_tensor(w4[:, 1:2], g1[:, 1:2], fr[:, 0:1], mult)
            nc.vector.tensor_tensor(w4[:, 2:3], fr[:, 1:2], g1[:, 0:1], mult)
            nc.vector.tensor_tensor(w4[:, 3:4], fr[:, 1:2], fr[:, 0:1], mult)

            # val[p] = sum of the 4 weighted taps
            tt = pool.tile([P, 4], f32)
            val = pool.tile([P, 1], f32)
            nc.vector.scalar_tensor_tensor(
                out=tt[:], in0=gv[:], scalar=1.0, in1=w4[:],
                op0=bypass, op1=mult, accum_out=val[:],
            )

            # loss[p] = softplus(sign[p] * val[p]) = ln(1 + exp(sign*val)).
            # bias/scale come from cb columns so no extra const DMAs appear.
            ez = pool.tile([P, 1], f32)
            nc.scalar.activation(
                out=ez[:], in_=val[:],
                func=mybir.ActivationFunctionType.Exp,
                bias=cb[:, 5:6], scale=cb[:, 2:3],
            )
            sp = pool.tile([P, 1], f32)
            nc.scalar.activation(
                out=sp[:], in_=ez[:],
                func=mybir.ActivationFunctionType.Ln,
                bias=cb[:, 4:5],
            )

            # partition sum via matmul against the 1/1024 column
            acc = psum.tile([1, 1], f32)
            nc.tensor.matmul(
                out=acc[:], lhsT=sp[:], rhs=cb[:, 3:4], start=True, stop=True
            )
            res = pool.tile([1, 1], f32)
            nc.vector.tensor_copy(res[:], acc[:])
            nc.sync.dma_start(out_d[:, :], res[:])

    tile.TileContext._drain_and_barrier = orig_dab
    _compile_with_joint_act_table(nc, bacc, mybir)
    return nc


def _slim_drain_and_barrier(tile_mod):
    """Kernel-exit: keep the drain (output DMA must land) + one all-engine
    barrier, but skip the end-of-kernel semaphore clear + second barrier -
    the Bass preamble re-clears all kernel semaphores at the start of every
    execution, so the exit clear only duplicates work on the measured path."""
    orig = tile_mod.TileContext._drain_and_barrier

    def slim(self, tick_clock, wait_clock):
        drain_inst = self.nc.sync.drain()
        wait_clock.add_sem_waits(
            drain_inst.ins, tile_mod.ScopedClock({None: tick_clock.global_clock})
        )
        self.nc.all_engine_barrier()
        popped = self.nc._tile_sem_poison_stack.pop()
        assert popped is self._sem_poison

    tile_mod.TileContext._drain_and_barrier = slim
    return orig


def _compile_with_joint_act_table(nc, bacc, mybir):
    """Compile with Exp and Ln forced into the one table set that holds both
    (natural_log_exp_and_others), avoiding a 1.3us ACT table reload between
    the two activations."""
    orig = bacc.get_activation_tables
    exp_t = mybir.ActivationFunctionType.Exp
    ln_t = mybir.ActivationFunctionType.Ln

    def patched(arch):
        tables = dict(orig(arch))
        joint = "natural_log_exp_and_others"
        if joint in tables and exp_t in tables[joint] and ln_t in tables[joint]:
            for name in tables:
                if name != joint:
                    tables[name] = tables[name] - {exp_t, ln_t}
        return tables

    bacc.get_activation_tables = patched
    try:
        nc.compile()
    finally:
        bacc.get_activation_tables = orig


def shard_inputs(pred_mask, positive_points, negative_points):
    pm = np.ascontiguousarray(
        np.asarray(pred_mask, dtype=np.float32).reshape(NCORES, BL * H * W, 1)
    )
    pos = np.asarray(positive_points, dtype=np.float32).reshape(NCORES, BL * NPOS, 2)
    neg = np.asarray(negative_points, dtype=np.float32).reshape(NCORES, BL * NNEG, 2)
    pts = np.ascontiguousarray(np.concatenate([pos, neg], axis=1))  # [8, 128, 2]
    return [{"pm": pm[c], "pts": pts[c]} for c in range(NCORES)]


def _harden_runtime():
    """Make run_bass_kernel_spmd robust in this container even if the caller
    enables tracing: the agent image's antenv lacks axon_hooks (crashes the
    trace path on import), and the S3 artifact upload has no credentials
    here.  Both shims are no-ops when tracing is off."""
    try:
        from antenv import axon_hooks  # noqa: F401
    except ImportError:
        import types

        import antenv

        mod = types.ModuleType("antenv.axon_hooks")
        _store = {}
        mod.set_axon_ntff_profile_hook = lambda h: _store.__setitem__("h", h)
        mod.get_axon_ntff_profile_hook = lambda: _store.get("h")
        sys.modules["antenv.axon_hooks"] = mod
        antenv.axon_hooks = mod
        try:
            from trn_agent_boot.trn_boot import _ntff_profile_via_ctypes

            mod.set_axon_ntff_profile_hook(
                _ntff_profile_via_ctypes("/opt/axon/libaxon_pjrt.so")
            )
        except Exception:
            pass

    from concourse import bass_utils

    if not getattr(bass_utils.upload_artifacts, "_failsafe", False):
        orig = bass_utils.upload_artifacts

        def safe_upload(tmpdir):
            try:
                return orig(tmpdir)
            except Exception:
                return f"file://{tmpdir}"

        safe_upload._failsafe = True
        bass_utils.upload_artifacts = safe_upload


def kernel(pred_mask, positive_points, negative_points):
    global _BUILT, LAST_RESULTS
    _harden_runtime()
    from concourse.bass_utils import run_bass_kernel_spmd

    if _BUILT is None:
        _BUILT = build()
    in_maps = shard_inputs(pred_mask, positive_points, negative_points)
    res = run_bass_kernel_spmd(_BUILT, in_maps, core_ids=list(range(NCORES)))
    LAST_RESULTS = res
    total = float(sum(float(r["out"][0, 0]) for r in res.results))
    return np.float32(total)


# revision 29
# speedup vs baseline: 1.2079x; 1.0009x over previous
"""PointClickLoss Trainium2 kernel.

Reference computes, for each of 32 images (1024x1024), bilinear samples at
16 positive + 16 negative points, BCE-with-logits losses via softplus, and
a mean over points/images.  Only 4 taps per point are actually needed from
the 128 MB pred_mask, so the kernel is a tiny data-dependent gather:

  - data parallel over 8 NeuronCores, 4 images each
  - per core: 128 points -> one partition each (pos points in partitions
    0..63, neg in 64..127; host concatenates the two point tensors)
  - tap indices computed on-device (floor via int32 roundtrip + compare
    correction; HW CAST rounds to nearest), then two indirect DMAs of 128
    descriptors x 8 bytes (one per tap row; the HW DGE emits one descriptor
    per offset-AP partition, sized by the dest row - probed on HW)
  - bilinear combine + softplus(+/-z)=ln(1+exp(+/-z)) on DVE/ACT (both in
    the natural_log_exp_and_others table set - no table reload),
    partition-sum via PE matmul against a 1/1024-scaled ones vector
  - each core emits its partial; host adds the 8 scalars
"""

import sys

if "/opt/trn_rl_repo" not in sys.path:
    sys.path.insert(0, "/opt/trn_rl_repo")

import numpy as np

B, H, W = 32, 1024, 1024
NPOS = NNEG = 16
NCORES = 8
BL = B // NCORES          # images per core
P = 128                   # partitions used = BL * (NPOS + NNEG)

_BUILT = None
LAST_RESULTS = None       # BassKernelResults of the most recent run (for test.py)


def build():
    import concourse.bass as bass
    import concourse.mybir as mybir
    import concourse.tile as tile
    from concourse import bacc

    f32 = mybir.dt.float32
    i32 = mybir.dt.int32
    mult = mybir.AluOpType.mult
    add = mybir.AluOpType.add
    bypass = mybir.AluOpType.bypass

    orig_dab = _slim_drain_and_barrier(tile)
    # The two all-engine barriers Bass.__init__ emits include per-engine
    # DRAINs (the Sync one costs ~0.7us scanning idle DMA rings).  Nothing
    # is in flight at model entry (NRT's own entry barrier precedes us), so
    # sequencer-level event sync is enough - keep the ordering, drop the
    # drains.
    orig_aeb = bass.Bass.all_engine_barrier
    ncalls = [0]

    def _sem_only_aeb(self, **kw):
        ncalls[0] += 1
        if ncalls[0] == 2:
            return  # second init barrier isolates host-side bookkeeping only
        return orig_aeb(self, sem_only=True)

    # Skip the four const-AP pool MEMSETs (const-float32-0.0 etc.) - this
    # kernel never reads them (bias/scale come from cb columns), and they
    # sit on Pool's serial path to the init barrier (~0.4us).
    orig_memset = bass.BassEitherVectorEngine.memset

    def _filtered_memset(self, ap, constant):
        name = getattr(ap.tensor, "name", "")
        if name.startswith("const-"):
            return None
        return orig_memset(self, ap, constant)

    bass.Bass.all_engine_barrier = _sem_only_aeb
    bass.BassEitherVectorEngine.memset = _filtered_memset
    try:
        nc = bacc.Bacc()
    finally:
        bass.Bass.all_engine_barrier = orig_aeb
        bass.BassEitherVectorEngine.memset = orig_memset
    pm = nc.dram_tensor("pm", [BL * H * W, 1], f32, kind="ExternalInput")
    pts_d = nc.dram_tensor("pts", [P, 2], f32, kind="ExternalInput")
    out_d = nc.dram_tensor("out", [1, 1], f32, kind="ExternalOutput")

    # Per-partition constants: partition p < 64 is positive point (p % 16) of
    # image p // 16; partition p >= 64 is negative point of image (p-64)//16.
    # cols: [flat base offset of the image, softplus sign, 1/1024, one, zero]
    pidx = np.arange(P)
    img = (pidx % 64) // 16
    base = (img * H * W).astype(np.float64)
    cb_np = np.stack(
        [
            base,
            base + W,
            np.where(pidx < 64, -1.0, 1.0),
            np.full(P, 1.0 / float(B * (NPOS + NNEG))),
            np.ones(P),
            np.zeros(P),
        ],
        axis=1,
    ).astype(np.float32)
    cb_d = nc.inline_tensor(cb_np, name="cb_const")

    with tile.TileContext(nc) as tc:
        with (
            tc.tile_pool(name="sb", bufs=1) as pool,
            tc.tile_pool(name="ps", bufs=1, space="PSUM") as psum,
        ):
            pts = pool.tile([P, 2], f32)          # (x, y) per point
            nc.sync.dma_start(pts[:, :], pts_d[:, :])
            cb = pool.tile([P, 6], f32)
            nc.scalar.dma_start(cb[:], cb_d[:, :])

            # --- critical index chain (4 DVE ops) ---
            # floor(x) = round_to_nearest(x - c) with c = largest f32 < 0.5.
            # HW casts round to nearest-even; exhaustively checked over
            # [0, 1023): result is floor(x), or floor(x)-1 exactly when the
            # frac weight comes out exactly 1.0 (bilinear then reads only the
            # in-range upper tap, so the sample is still exact).  CoreSim
            # truncates instead - sim_check models that separately.
            with tc.high_priority():
                xi = pool.tile([P, 2], i32)
                nc.vector.tensor_scalar(
                    out=xi[:], in0=pts[:], scalar1=-0.49999997, scalar2=None,
                    op0=add,
                )
                xy0 = pool.tile([P, 2], f32)      # (x0, y0) as floats (exact)
                nc.vector.tensor_copy(xy0[:], xi[:])
                r = pool.tile([P, 1], f32)        # y0*W + x0  (exact in f32)
                nc.vector.scalar_tensor_tensor(
                    out=r[:], in0=xy0[:, 1:2], scalar=float(W), in1=xy0[:, 0:1],
                    op0=mult, op1=add,
                )
                idx = pool.tile([P, 2], i32)      # +image base / +base+W
                nc.vector.scalar_tensor_tensor(
                    out=idx[:], in0=cb[:, 0:2], scalar=r[:, 0:1], in1=cb[:, 0:2],
                    op0=add, op1=bypass,
                )

            # Gather the 2x2 taps: one descriptor per partition per DMA, each
            # reading 2 contiguous f32 -> gv = (v00, v01, v10, v11).  The HW
            # DGE takes one offset per offset-AP *partition* sized by the dest
            # row (probed), so the two tap rows need two indirect DMAs.
            gv = pool.tile([P, 4], f32)
            nc.gpsimd.indirect_dma_start(
                out=gv[:, 0:2],
                out_offset=None,
                in_=pm[:, :],
                in_offset=bass.IndirectOffsetOnAxis(ap=idx[:, 0:1], axis=0),
            )
            nc.gpsimd.indirect_dma_start(
                out=gv[:, 2:4],
                out_offset=None,
                in_=pm[:, :],
                in_offset=bass.IndirectOffsetOnAxis(ap=idx[:, 1:2], axis=0),
            )

            # --- off-critical weight math (overlaps the gather) ---
            fr = pool.tile([P, 2], f32)           # (wx1, wy1)
            nc.vector.tensor_sub(fr[:], pts[:], xy0[:])
            g1 = pool.tile([P, 2], f32)           # (wx0, wy0) = 1 - frac
            nc.vector.scalar_tensor_tensor(
                out=g1[:], in0=xy0[:], scalar=1.0, in1=pts[:],
                op0=add, op1=mybir.AluOpType.subtract,
            )
            w4 = pool.tile([P, 4], f32)           # (wy0wx0, wy0wx1, wy1wx0, wy1wx1)
            nc.vector.tensor_tensor(w4[:, 0:1], g1[:, 1:2], g1[:, 0:1], mult)
            nc.vector.tensor_tensor(w4[:, 1:2], g1[:, 1:2], fr[:, 0:1], mult)
            nc.vector.tensor_tensor(w4[:, 2:3], fr[:, 1:2], g1[:, 0:1], mult)
            nc.vector.tensor_tensor(w4[:, 3:4], fr[:, 1:2], fr[:, 0:1], mult)

            # val[p] = sum of the 4 weighted taps
            tt = pool.tile([P, 4], f32)
            val = pool.tile([P, 1], f32)
            nc.vector.scalar_tensor_tensor(
                out=tt[:], in0=gv[:], scalar=1.0, in1=w4[:],
                op0=bypass, op1=mult, accum_out=val[:],
            )

            # loss[p] = softplus(sign[p] * val[p]) = ln(1 + exp(sign*val)).
            # bias/scale come from cb columns so no extra const DMAs appear.
            ez = pool.tile([P, 1], f32)
            nc.scalar.activation(
                out=ez[:], in_=val[:],
                func=mybir.ActivationFunctionType.Exp,
                bias=cb[:, 5:6], scale=cb[:, 2:3],
            )
            sp = pool.tile([P, 1], f32)
            nc.scalar.activation(
                out=sp[:], in_=ez[:],
                func=mybir.ActivationFunctionType.Ln,
                bias=cb[:, 4:5],
            )

            # partition sum via matmul against the 1/1024 column
            acc = psum.tile([1, 1], f32)
            nc.tensor.matmul(
                out=acc[:], lhsT=sp[:], rhs=cb[:, 3:4], start=True, stop=True
            )
            res = pool.tile([1, 1], f32)
            nc.vector.tensor_copy(res[:], acc[:])
            nc.sync.dma_start(out_d[:, :], res[:])

    tile.TileContext._drain_and_barrier = orig_dab
    _compile_with_joint_act_table(nc, bacc, mybir)
    return nc


def _slim_drain_and_barrier(tile_mod):
    """Kernel-exit: keep the drain (output DMA must land) + one all-engine
    barrier, but skip the end-of-kernel semaphore clear + second barrier -
    the Bass preamble re-clears all kernel semaphores at the start of every
    execution, so the exit clear only duplicates work on the measured path."""
    orig = tile_mod.TileContext._drain_and_barrier

    def slim(self, tick_clock, wait_clock):
        # The Sync drain waits for every DMA completion sem (incl. the output
        # store) - that is the part that must gate model end.  The other
        # engines' work is all upstream of that DMA, so event-only sync
        # suffices for them.
        drain_inst = self.nc.sync.drain()
        wait_clock.add_sem_waits(
            drain_inst.ins, tile_mod.ScopedClock({None: tick_clock.global_clock})
        )
        self.nc.all_engine_barrier(sem_only=True)
        popped = self.nc._tile_sem_poison_stack.pop()
        assert popped is self._sem_poison

    tile_mod.TileContext._drain_and_barrier = slim
    return orig


def _compile_with_joint_act_table(nc, bacc, mybir):
    """Compile with Exp and Ln forced into the one table set that holds both
    (natural_log_exp_and_others), avoiding a 1.3us ACT table reload between
    the two activations."""
    orig = bacc.get_activation_tables
    exp_t = mybir.ActivationFunctionType.Exp
    ln_t = mybir.ActivationFunctionType.Ln

    def patched(arch):
        tables = dict(orig(arch))
        joint = "natural_log_exp_and_others"
        if joint in tables and exp_t in tables[joint] and ln_t in tables[joint]:
            for name in tables:
                if name != joint:
                    tables[name] = tables[name] - {exp_t, ln_t}
        return tables

    bacc.get_activation_tables = patched
    try:
        nc.compile()
    finally:
        bacc.get_activation_tables = orig


def shard_inputs(pred_mask, positive_points, negative_points):
    pm = np.ascontiguousarray(
        np.asarray(pred_mask, dtype=np.float32).reshape(NCORES, BL * H * W, 1)
    )
    pos = np.asarray(positive_points, dtype=np.float32).reshape(NCORES, BL * NPOS, 2)
    neg = np.asarray(negative_points, dtype=np.float32).reshape(NCORES, BL * NNEG, 2)
    pts = np.ascontiguousarray(np.concatenate([pos, neg], axis=1))  # [8, 128, 2]
    return [{"pm": pm[c], "pts": pts[c]} for c in range(NCORES)]


def _harden_runtime():
    """Make run_bass_kernel_spmd robust in this container even if the caller
    enables tracing: the agent image's antenv lacks axon_hooks (crashes the
    trace path on import), and the S3 artifact upload has no credentials
    here.  Both shims are no-ops when tracing is off."""
    try:
        from antenv import axon_hooks  # noqa: F401
    except ImportError:
        import types

        import antenv

        mod = types.ModuleType("antenv.axon_hooks")
        _store = {}
        mod.set_axon_ntff_profile_hook = lambda h: _store.__setitem__("h", h)
        mod.get_axon_ntff_profile_hook = lambda: _store.get("h")
        sys.modules["antenv.axon_hooks"] = mod
        antenv.axon_hooks = mod
        try:
            from trn_agent_boot.trn_boot import _ntff_profile_via_ctypes

            mod.set_axon_ntff_profile_hook(
                _ntff_profile_via_ctypes("/opt/axon/libaxon_pjrt.so")
            )
        except Exception:
            pass

    from concourse import bass_utils

    if not getattr(bass_utils.upload_artifacts, "_failsafe", False):
        orig = bass_utils.upload_artifacts

        def safe_upload(tmpdir):
            try:
                return orig(tmpdir)
            except Exception:
                return f"file://{tmpdir}"

        safe_upload._failsafe = True
        bass_utils.upload_artifacts = safe_upload


def kernel(pred_mask, positive_points, negative_points):
    global _BUILT, LAST_RESULTS
    _harden_runtime()
    from concourse.bass_utils import run_bass_kernel_spmd

    if _BUILT is None:
        _BUILT = build()
    in_maps = shard_inputs(pred_mask, positive_points, negative_points)
    res = run_bass_kernel_spmd(_BUILT, in_maps, core_ids=list(range(NCORES)))
    LAST_RESULTS = res
    total = float(sum(float(r["out"][0, 0]) for r in res.results))
    return np.float32(total)
